# revision 39
# baseline (speedup 1.0000x reference)
"""Multi-head causal attention (B=2, T=2048, D=1024, H=16) on 8 trn2 NeuronCores.

Sharding: 8 cores = 2 batches x 4 head-groups (4 heads each). Each core:
  - computes qkv projections for its 4 heads from x[b] (pre-transposed on host),
  - runs masked softmax attention in transposed (k, q) score layout,
  - emits a partial output projection y_part = attn_heads @ w_out[head_rows].
Host sums the 4 partial y per batch (partials emitted in bf16).

All matmuls run in bf16 with fp32 PSUM accumulation. Softmax skips the
max-subtraction (scores are ~N(0,1)): one merged exp per (k,q) block covers
both heads of a pair via a two-bank PSUM tile; softmax denominators come for
free from an all-ones block appended to v (matmul cost depends only on the
moving dim), and a single full-width reciprocal per (pair, q-tile) serves
both heads. Attention q/k/out projections and the output projection are
interleaved into the exp-paced attention loop as PE filler work; the two
head-pairs alternate per q-tile so out-projections spread across the run,
and the last q-tiles' out-projections are deferred to overlap the final
normalization chain (whose two half-swap DMAs dispatch from different
engine queues to parallelize descriptor generation). Attention accumulators
are evicted to SBUF immediately after the last accumulation to free PSUM
banks for the next pair. Mask multiplies run on the otherwise-idle GpSimd
engine, keeping the exp->attnU dependency off the in-order DVE queue.
A warmup matmul burst during the initial DMA window ramps the PE p-state.
Mask handling is generic: the host classifies (128k x 512q) blocks of the
provided mask into skip / full / partial; partial blocks get a narrowed exp
plus a mask multiply.
"""
import sys
sys.path.insert(0, "/opt/trn_rl_repo")

import numpy as np
import ml_dtypes

import concourse.bass as bass
import concourse.mybir as mybir
import concourse.tile as tile
from concourse import bacc
from concourse.bass_utils import run_bass_kernel_spmd

B, T, D, H, Dh = 2, 2048, 1024, 16, 64
P = 128
QT = 512              # q-tile width (score tile free dim)
NQ = T // QT          # 4
NKT = T // P          # 16
ND = D // P           # 8
HPC = 4               # heads per core
NPAIR = HPC // 2      # head pairs per core
N_CORES = 8
N_WARM = 12           # PE p-state warmup matmuls during the DMA window

f32 = mybir.dt.float32
bf16 = mybir.dt.bfloat16
CDT = bf16            # compute dtype for matmul operands
NP_CDT = ml_dtypes.bfloat16


def _block_structure(mask: np.ndarray):
    """Classify maskT (k,q) blocks: per q-tile a list of (kt, pattern_idx|None).

    For each unique partial pattern also derive (w0, m_lo, m_hi): w0 leading
    all-masked columns (exp skipped, memset 0), and [m_lo, m_hi) the column
    range that still needs the mask multiply.
    """
    maskT = (mask != 0).T.astype(np.float32)  # [k, q] visibility
    vis = []
    patterns = []
    meta = []
    pat_index = {}
    for qt in range(NQ):
        row = []
        for kt in range(NKT):
            blk = maskT[kt * P:(kt + 1) * P, qt * QT:(qt + 1) * QT]
            s = blk.sum()
            if s == 0:
                continue
            if s == blk.size:
                row.append((kt, None))
            else:
                key = blk.tobytes()
                if key not in pat_index:
                    pat_index[key] = len(patterns)
                    patterns.append(blk)
                    col_any = blk.any(axis=0)       # column has any visible
                    col_all = blk.all(axis=0)       # column fully visible
                    w0 = int(np.argmax(col_any)) if col_any.any() else QT
                    partial_cols = np.nonzero(col_any & ~col_all)[0]
                    if partial_cols.size:
                        m_lo, m_hi = int(partial_cols[0]), int(partial_cols[-1]) + 1
                    else:
                        m_lo = m_hi = 0
                    meta.append((w0, m_lo, m_hi))
                row.append((kt, pat_index[key]))
        vis.append(row)
    if patterns:
        pm = np.stack(patterns)
    else:
        pm = np.zeros((1, P, QT), np.float32)
    return vis, pm, meta


def _build_program(vis, n_pm, meta=(), compile=True):
    nc = bacc.Bacc() if compile else bass.Bass()
    # inputs are host-prepacked into the on-chip [partition, ...] layout so
    # every DMA moves long contiguous rows (4-16KB descriptors instead of 512B)
    xc0a = nc.declare_dram_parameter("xc0a", [P, ND, 256], CDT, isOutput=False)
    xc0b = nc.declare_dram_parameter("xc0b", [P, ND, 256], CDT, isOutput=False)
    xc1 = nc.declare_dram_parameter("xc1", [P, ND, 512], CDT, isOutput=False)
    xc2 = nc.declare_dram_parameter("xc2", [P, ND, 1024], CDT, isOutput=False)
    wqk = nc.declare_dram_parameter("wqk", [P, ND, 2 * HPC * Dh], CDT, isOutput=False)
    wv = nc.declare_dram_parameter("wv", [P, ND, HPC * Dh], CDT, isOutput=False)
    wo = nc.declare_dram_parameter("wo", [P, NPAIR, D], CDT, isOutput=False)
    pmask = nc.declare_dram_parameter("pmask", [P, n_pm, QT], CDT, isOutput=False)
    y = nc.declare_dram_parameter("y", [T, D], CDT, isOutput=True)

    inv_sqrt_dh = 1.0 / float(np.sqrt(Dh))

    with tile.TileContext(nc) as tc:
        with (
            tc.tile_pool(name="persist", bufs=1) as persist,
            tc.tile_pool(name="work", bufs=3) as work,
            tc.tile_pool(name="psA", bufs=2, space="PSUM") as psA,
            tc.tile_pool(name="psS", bufs=2, space="PSUM") as psS,
            tc.tile_pool(name="psU", bufs=1, space="PSUM") as psU,
        ):
            # ---- persistent SBUF tensors ----
            xt_c = [persist.tile([P, ND, 512], CDT, tag="xt0", name="xt0"),
                    persist.tile([P, ND, 512], CDT, tag="xt1", name="xt1"),
                    persist.tile([P, ND, 1024], CDT, tag="xt2", name="xt2")]

            def xt_ap(dt, a, b):
                if b <= 512:
                    return xt_c[0][:, dt, a:b]
                if b <= 1024:
                    return xt_c[1][:, dt, a - 512:b - 512]
                return xt_c[2][:, dt, a - 1024:b - 1024]

            wqk_sb = persist.tile([P, ND, 2 * HPC * Dh], CDT, tag="wqk")
            wv_sb = persist.tile([P, ND, HPC * Dh], CDT, tag="wv")
            wo_sb = persist.tile([P, NPAIR, D], CDT, tag="wo")
            pm_sb = persist.tile([P, n_pm, QT], CDT, tag="pm")
            scratch = persist.tile([P, QT], CDT, tag="scratch")
            # per-pair tensors (separate tiles so cross-pair interleaving
            # cannot create false dependencies)
            qT_sb = [persist.tile([P, T], CDT, tag=f"qT{p}", name=f"qT{p}") for p in range(NPAIR)]
            kT_sb = [persist.tile([P, T], CDT, tag=f"kT{p}", name=f"kT{p}") for p in range(NPAIR)]
            # at tiles are split per (pair, q-tile) so deferred out-projection
            # reads never serialize behind later q-tiles' normalization writes
            at_sb = [[persist.tile([P, QT], CDT, tag=f"at{p}q{q}", name=f"at{p}q{q}")
                      for q in range(NQ)] for p in range(NPAIR)]
            # v1: per k-tile and head, [128, 128]: for even heads cols 0:64 =
            # v values and cols 64:128 all-ones (for odd heads the reverse),
            # so the attnU matmul emits softmax denominators replicated on the
            # complementary partition half (matmul cost only depends on N).
            v1_sb = persist.tile([P, NKT, HPC, P], CDT, tag="v1")

            # ---- PE p-state warmup: garbage matmuls with no DMA deps ----
            nc.vector.memset(scratch[:], 0.0)
            ps_warm = psA.tile([P, QT], f32, tag="psA", name="warm")
            for i in range(N_WARM):
                nc.tensor.matmul(
                    ps_warm[:],
                    scratch[:, 0:P],
                    scratch[:],
                    start=(i == 0),
                    stop=(i == N_WARM - 1),
                )

            # ones blocks only (v halves written by emit_v)
            nc.vector.memset(v1_sb[:, :, 0::2, Dh:P], 1.0)
            nc.vector.memset(v1_sb[:, :, 1::2, 0:Dh], 1.0)

            # ---- DMAs in criticality order; flat APs so rows coalesce ----
            def flat(ap):
                return ap.rearrange("p a b -> p (a b)")

            nc.sync.dma_start(flat(wv_sb[:]), flat(wv[:]))
            nc.sync.dma_start(xt_c[0][:, :, 0:256], xc0a[:])
            nc.sync.dma_start(xt_c[0][:, :, 256:512], xc0b[:])
            nc.sync.dma_start(flat(wqk_sb[:]), flat(wqk[:]))
            nc.sync.dma_start(flat(pm_sb[:]), flat(pmask[:]))
            nc.sync.dma_start(flat(xt_c[1][:]), flat(xc1[:]))
            nc.sync.dma_start(flat(xt_c[2][:]), flat(xc2[:]))
            nc.sync.dma_start(flat(wo_sb[:]), flat(wo[:]))

            # ---- filler queue: PE-side work interleaved into ACT-paced ----
            # ---- attention steps                                        ----
            fillers = []  # list of (key, thunk); emitted in order
            late = []     # out-projections reserved to overlap the tail

            def drain(k):
                for _ in range(min(k, len(fillers))):
                    fillers.pop(0)[1]()

            def flush_through(pred):
                """Emit queued fillers (in order) until none matching pred remain."""
                while any(pred(key) for key, _ in fillers):
                    fillers.pop(0)[1]()

            # ---- phase A: v = x @ wv ----
            def emit_v(tt):
                ps_v = psA.tile([P, QT], f32, tag="psA", name=f"psv{tt}")
                for dt in range(ND):
                    nc.tensor.matmul(
                        ps_v[:, :HPC * Dh],
                        xt_ap(dt, tt * P, (tt + 1) * P),
                        wv_sb[:, dt, :],
                        start=(dt == 0),
                        stop=(dt == ND - 1),
                    )
                ps_vh = ps_v[:, :HPC * Dh].rearrange("p (h e) -> p h e", h=HPC)
                nc.vector.tensor_copy(v1_sb[:, tt, 0::2, 0:Dh], ps_vh[:, 0::2])
                nc.vector.tensor_copy(v1_sb[:, tt, 1::2, Dh:P], ps_vh[:, 1::2])

            # ---- phase A: qT / kT for pair p, one (tensor, nt) at a time ----
            def qk_units(p, nt):
                units = []
                # pair-major weight layout: [wq_p0 | wk_p0 | wq_p1 | wk_p1]
                for w_off, out_sb in ((p * 2 * P, qT_sb[p]), (p * 2 * P + P, kT_sb[p])):
                    ps_box = []

                    def mm(dt, w_off=w_off, nt=nt, ps_box=ps_box, p=p):
                        if dt == 0:
                            ps_box.append(psA.tile(
                                [P, QT], f32, tag="psA", name=f"psqk{p}_{nt}_{w_off}"))
                        nc.tensor.matmul(
                            ps_box[0],
                            wqk_sb[:, dt, w_off:w_off + P],
                            xt_ap(dt, nt * QT, (nt + 1) * QT),
                            start=(dt == 0),
                            stop=(dt == ND - 1),
                        )

                    def evict(out_sb=out_sb, nt=nt, ps_box=ps_box):
                        nc.vector.tensor_copy(
                            out_sb[:, nt * QT:(nt + 1) * QT], ps_box[0])

                    key = ("qk", p, nt)

                    def mk(dt, mm=mm):
                        return lambda: mm(dt)

                    units.extend((key, mk(dt)) for dt in range(ND))
                    units.append((key, evict))
                return units

            # ---- phase C: out-projection for one t-tile/half (as filler) ----
            def make_outproj(tt, half):
                def go():
                    ps_y = psA.tile([P, QT], f32, tag="psA", name=f"psy{tt}_{half}")
                    qi, ci = tt // (QT // P), (tt % (QT // P)) * P
                    for p in range(NPAIR):
                        nc.tensor.matmul(
                            ps_y[:],
                            at_sb[p][qi][:, ci:ci + P],
                            wo_sb[:, p, half * QT:(half + 1) * QT],
                            start=(p == 0),
                            stop=(p == NPAIR - 1),
                        )
                    ysb = work.tile([P, QT], CDT, tag="y", name=f"y{tt}_{half}")
                    # tail-reserved units evict via the Activation engine: at
                    # the tail the DVE queue is head-of-line blocked on the
                    # normalization chain, which would stall psA bank reuse
                    nc.vector.tensor_copy(ysb[:], ps_y[:])
                    nc.sync.dma_start(
                        y[tt * P:(tt + 1) * P, half * QT:(half + 1) * QT], ysb[:])
                return go

            # ---- phase B: attention for pair p, q-tile qt (sw-pipelined) ----
            def emit_attention(p, qt):
                row = vis[qt]
                max_kt = max((kt for kt, _ in row), default=-1)
                flush_through(lambda key: (
                    (key[0] == "v" and key[1] <= max_kt)
                    or (key[0] == "qk" and key[1] == p and key[2] == qt)))
                if not row:
                    for h in range(2):
                        nc.vector.memset(at_sb[p][qt][h * Dh:(h + 1) * Dh, :], 0.0)
                    return
                ps_u = [
                    psU.tile([P, QT], f32, tag=f"u{h}", name=f"u{h}_{p}_{qt}")
                    for h in range(2)
                ]
                es_q = []

                def emit_scores(j, p=p, qt=qt, row=row, es_q=es_q):
                    kt, pidx = row[j]
                    w0 = 0 if pidx is None else meta[pidx][0]
                    if j == 0:
                        w0 = 0  # first accumulation must set has_written
                    # both heads' scores into one two-bank psum tile; the two
                    # matmuls run concurrently on complementary PE tile rows
                    ps_s = psS.tile([P, 2, QT], f32, tag="s", name=f"s_{p}_{qt}_{kt}")
                    for h in range(2):
                        base = h * Dh
                        nc.tensor.matmul(
                            ps_s[:, h, w0:QT],
                            kT_sb[p][base:base + Dh, kt * P:(kt + 1) * P],
                            qT_sb[p][base:base + Dh, qt * QT + w0:(qt + 1) * QT],
                            start=True,
                            stop=True,
                            tile_position=(base, 0),
                        )
                    es = work.tile([P, 2, QT], CDT, tag="es", bufs=5, name=f"es_{p}_{qt}_{kt}")
                    nc.scalar.activation(
                        es[:, :, w0:QT], ps_s[:, :, w0:QT],
                        mybir.ActivationFunctionType.Exp,
                        scale=inv_sqrt_dh,
                    )
                    if pidx is not None:
                        _w0, m_lo, m_hi = meta[pidx]
                        m_lo = min(m_lo, w0)  # w0 forced to 0 on j==0
                        if m_hi > m_lo:
                            # run the mask multiplies on the otherwise-idle
                            # GpSimd engine: keeps the exp->attnU dependency
                            # off the in-order DVE queue
                            for h in range(2):
                                nc.gpsimd.tensor_mul(
                                    es[:, h, m_lo:m_hi], es[:, h, m_lo:m_hi],
                                    pm_sb[:, pidx, m_lo:m_hi],
                                )
                    es_q.append((es, w0))

                def emit_attnu(j, p=p, row=row, es_q=es_q, ps_u=ps_u):
                    kt, _ = row[j]
                    es, w0 = es_q[j]
                    for h in range(2):
                        nc.tensor.matmul(
                            ps_u[h][:, w0:QT],
                            v1_sb[:, kt, 2 * p + h, :],
                            es[:, h, w0:QT],
                            start=(j == 0),
                            stop=(j == len(row) - 1),
                        )

                # process blocks in batches of two so the PE array switches
                # between tiled mode (64-row scores pairs) and full mode
                # (attnU + fillers) half as often — each mode switch exposes
                # a ~100ns weight-load/drain.  Fillers run on the full-mode
                # side of each batch.
                emit_scores(0)
                if len(row) > 1:
                    emit_scores(1)
                for b0 in range(0, len(row), 2):
                    cur = [j for j in (b0, b0 + 1) if j < len(row)]
                    nxt = [j for j in (b0 + 2, b0 + 3) if j < len(row)]
                    drain(2 * len(cur))
                    for j in nxt:
                        emit_scores(j)
                    for j in cur:
                        emit_attnu(j)

                # normalization: h0 denom replicated at psum partitions 64:128
                # of ps_u[0], h1 denom at partitions 0:64 of ps_u[1].  Evict
                # both psum tiles to SBUF immediately (frees psU for the next
                # pair; DVE copy cost is free-dim-bound so full-width copies
                # cost the same as halves), swap denominator halves via
                # sbuf-to-sbuf DMA, one full-width reciprocal, then one
                # aligned multiply per head.
                s0 = work.tile([P, QT], f32, tag="s0", name=f"s0_{p}_{qt}")
                s1 = work.tile([P, QT], f32, tag="s1", name=f"s1_{p}_{qt}")
                nc.vector.tensor_copy(s0[:], ps_u[0][:])
                nc.vector.tensor_copy(s1[:], ps_u[1][:])
                comb = work.tile([P, QT], f32, tag="comb", name=f"comb_{p}_{qt}")
                # dispatch the two half-swaps from different engine queues so
                # their descriptor-generation does not serialize
                nc.sync.dma_start(comb[0:Dh, :], s0[Dh:P, :])
                nc.scalar.dma_start(comb[Dh:P, :], s1[0:Dh, :])
                rep = work.tile([P, QT], f32, tag="rep", name=f"rep_{p}_{qt}")
                nc.vector.reciprocal_approx_fast(rep[:], comb[:])
                nc.vector.tensor_mul(at_sb[p][qt][0:Dh, :], s0[0:Dh, :], rep[0:Dh, :])
                nc.vector.tensor_mul(at_sb[p][qt][Dh:P, :], s1[Dh:P, :], rep[Dh:P, :])
                if p == NPAIR - 1:
                    # out-projection for the t-tiles this qt completed; the
                    # last two q-tiles' units are reserved so the PE has work
                    # overlapping the final normalization chain + teardown
                    _late = qt >= NQ - 3
                    dst = late if _late else fillers
                    dst.extend(
                        (("op", tt, half), make_outproj(tt, half))
                        for tt in range(qt * (QT // P), (qt + 1) * (QT // P))
                        for half in range(2)
                    )

            # inline prologue: just enough for attention(p0, qt0) to start
            for tt in range(NQ):
                emit_v(tt)
            for nt in range(NQ):
                for p in range(NPAIR):
                    fillers.extend(qk_units(p, nt))
                if nt + 1 < NQ:
                    lo, hi = (nt + 1) * NQ, (nt + 2) * NQ
                    fillers.extend(
                        (("v", tt), (lambda tt=tt: emit_v(tt)))
                        for tt in range(lo, min(hi, NKT)))
            for qt in range(NQ):
                for p in range(NPAIR):
                    emit_attention(p, qt)
            while fillers:
                drain(len(fillers))
            for _, thunk in late:
                thunk()
    if compile:
        nc.compile()
    return nc


def _host_inputs(x, mask, w_qkv, w_out):
    vis, pm, meta = _block_structure(np.asarray(mask))
    pm_c = pm.astype(NP_CDT)
    wq_f, wk_f, wv_f = np.split(np.asarray(w_qkv, np.float32), 3, axis=1)
    in_maps = []
    for core in range(N_CORES):
        b = core // 4
        g = core % 4
        cols = slice(g * HPC * Dh, (g + 1) * HPC * Dh)
        wq_c, wk_c = wq_f[:, cols], wk_f[:, cols]
        wqk_c = np.concatenate(
            [wq_c[:, 0:2 * Dh], wk_c[:, 0:2 * Dh],
             wq_c[:, 2 * Dh:], wk_c[:, 2 * Dh:]], axis=1)

        def pack_d(a):  # [D, E] -> [P, ND, E] matching d = o*P + p
            return np.ascontiguousarray(
                a.reshape(ND, P, -1).transpose(1, 0, 2)).astype(NP_CDT)

        xp = pack_d(np.ascontiguousarray(np.asarray(x[b], np.float32).T))
        in_maps.append({
            "xc0a": np.ascontiguousarray(xp[:, :, 0:256]),
            "xc0b": np.ascontiguousarray(xp[:, :, 256:512]),
            "xc1": np.ascontiguousarray(xp[:, :, 512:1024]),
            "xc2": np.ascontiguousarray(xp[:, :, 1024:2048]),
            "wqk": pack_d(wqk_c),
            "wv": pack_d(wv_f[:, cols]),
            "wo": np.ascontiguousarray(
                np.asarray(w_out, np.float32)[cols, :]
                .reshape(NPAIR, P, D).transpose(1, 0, 2)).astype(NP_CDT),
            "pmask": np.ascontiguousarray(pm_c.transpose(1, 0, 2)),
        })
    return vis, pm, meta, in_maps


def run(x, mask, w_qkv, w_out, trace=False):
    import os
    vis, pm, meta, in_maps = _host_inputs(x, mask, w_qkv, w_out)
    nc = _build_program(vis, pm.shape[0], meta)
    if not trace:
        # An inherited BASS_TRACE=1 would pull in NTFF profiling hooks that
        # may not exist in this environment; force tracing off.
        os.environ["BASS_NEVER_TRACE"] = "1"
    else:
        os.environ.pop("BASS_NEVER_TRACE", None)
    res = run_bass_kernel_spmd(nc, in_maps, core_ids=list(range(N_CORES)), trace=trace)
    parts = [res.results[i]["y"].astype(np.float32) for i in range(N_CORES)]
    out = np.stack([
        parts[0] + parts[1] + parts[2] + parts[3],
        parts[4] + parts[5] + parts[6] + parts[7],
    ]).astype(np.float32)
    return out, res


def kernel(x, mask, w_qkv, w_out):
    out, _ = run(x, mask, w_qkv, w_out, trace=False)
    return out


# revision 40
# speedup vs baseline: 1.1892x; 1.1892x over previous
"""Multi-head causal attention (B=2, T=2048, D=1024, H=16) on 8 trn2 NeuronCores.

Sharding: 8 cores = 2 batches x 4 head-groups (4 heads each). Each core:
  - computes qkv projections for its 4 heads from x[b] (pre-transposed on host),
  - runs masked softmax attention in transposed (k, q) score layout,
  - emits a partial output projection y_part = attn_heads @ w_out[head_rows].
Host sums the 4 partial y per batch (partials emitted in bf16).

All matmuls run in bf16 with fp32 PSUM accumulation. Softmax skips the
max-subtraction (scores are ~N(0,1)): one merged exp per (k,q) block covers
both heads of a pair via a two-bank PSUM tile; softmax denominators come for
free from an all-ones block appended to v (matmul cost depends only on the
moving dim), and a single full-width reciprocal per (pair, q-tile) serves
both heads. Attention q/k/out projections and the output projection are
interleaved into the exp-paced attention loop as PE filler work; the two
head-pairs alternate per q-tile so out-projections spread across the run,
and the last q-tiles' out-projections are deferred to overlap the final
normalization chain (whose two half-swap DMAs dispatch from different
engine queues to parallelize descriptor generation). Attention accumulators
are evicted to SBUF immediately after the last accumulation to free PSUM
banks for the next pair. Mask multiplies run on the otherwise-idle GpSimd
engine, keeping the exp->attnU dependency off the in-order DVE queue.
A warmup matmul burst during the initial DMA window ramps the PE p-state.
Mask handling is generic: the host classifies (128k x 512q) blocks of the
provided mask into skip / full / partial; partial blocks get a narrowed exp
plus a mask multiply.
"""
import sys
sys.path.insert(0, "/opt/trn_rl_repo")

import numpy as np
import ml_dtypes

import concourse.bass as bass
import concourse.mybir as mybir
import concourse.tile as tile
from concourse import bacc
from concourse.bass_utils import run_bass_kernel_spmd

B, T, D, H, Dh = 2, 2048, 1024, 16, 64
P = 128
QT = 512              # q-tile width (score tile free dim)
NQ = T // QT          # 4
NKT = T // P          # 16
ND = D // P           # 8
HPC = 4               # heads per core
NPAIR = HPC // 2      # head pairs per core
N_CORES = 8
N_WARM = 12           # PE p-state warmup matmuls during the DMA window

f32 = mybir.dt.float32
bf16 = mybir.dt.bfloat16
CDT = bf16            # compute dtype for matmul operands
NP_CDT = ml_dtypes.bfloat16


def _block_structure(mask: np.ndarray):
    """Classify maskT (k,q) blocks: per q-tile a list of (kt, pattern_idx|None).

    For each unique partial pattern also derive (w0, m_lo, m_hi): w0 leading
    all-masked columns (exp skipped, memset 0), and [m_lo, m_hi) the column
    range that still needs the mask multiply.
    """
    maskT = (mask != 0).T.astype(np.float32)  # [k, q] visibility
    vis = []
    patterns = []
    meta = []
    pat_index = {}
    for qt in range(NQ):
        row = []
        for kt in range(NKT):
            blk = maskT[kt * P:(kt + 1) * P, qt * QT:(qt + 1) * QT]
            s = blk.sum()
            if s == 0:
                continue
            if s == blk.size:
                row.append((kt, None))
            else:
                key = blk.tobytes()
                if key not in pat_index:
                    pat_index[key] = len(patterns)
                    patterns.append(blk)
                    col_any = blk.any(axis=0)       # column has any visible
                    col_all = blk.all(axis=0)       # column fully visible
                    w0 = int(np.argmax(col_any)) if col_any.any() else QT
                    partial_cols = np.nonzero(col_any & ~col_all)[0]
                    if partial_cols.size:
                        m_lo, m_hi = int(partial_cols[0]), int(partial_cols[-1]) + 1
                    else:
                        m_lo = m_hi = 0
                    meta.append((w0, m_lo, m_hi))
                row.append((kt, pat_index[key]))
        vis.append(row)
    if patterns:
        pm = np.stack(patterns)
    else:
        pm = np.zeros((1, P, QT), np.float32)
    return vis, pm, meta


def _build_program(vis, n_pm, meta=(), compile=True):
    nc = bacc.Bacc() if compile else bass.Bass()
    # inputs are host-prepacked into the on-chip [partition, ...] layout so
    # every DMA moves long contiguous rows (4-16KB descriptors instead of 512B)
    xc0a = nc.declare_dram_parameter("xc0a", [P, ND, 256], CDT, isOutput=False)
    xc0b = nc.declare_dram_parameter("xc0b", [P, ND, 256], CDT, isOutput=False)
    xc1 = nc.declare_dram_parameter("xc1", [P, ND, 512], CDT, isOutput=False)
    xc2 = nc.declare_dram_parameter("xc2", [P, ND, 1024], CDT, isOutput=False)
    wqk = nc.declare_dram_parameter("wqk", [P, ND, 2 * HPC * Dh], CDT, isOutput=False)
    wv = nc.declare_dram_parameter("wv", [P, ND, HPC * Dh], CDT, isOutput=False)
    wo = nc.declare_dram_parameter("wo", [P, NPAIR, D], CDT, isOutput=False)
    pmask = nc.declare_dram_parameter("pmask", [P, n_pm, QT], CDT, isOutput=False)
    y = nc.declare_dram_parameter("y", [T, D], CDT, isOutput=True)

    inv_sqrt_dh = 1.0 / float(np.sqrt(Dh))

    with tile.TileContext(nc) as tc:
        with (
            tc.tile_pool(name="persist", bufs=1) as persist,
            tc.tile_pool(name="work", bufs=3) as work,
            tc.tile_pool(name="psA", bufs=2, space="PSUM") as psA,
            tc.tile_pool(name="psS", bufs=2, space="PSUM") as psS,
            tc.tile_pool(name="psU", bufs=1, space="PSUM") as psU,
        ):
            # ---- persistent SBUF tensors ----
            xt_c = [persist.tile([P, ND, 512], CDT, tag="xt0", name="xt0"),
                    persist.tile([P, ND, 512], CDT, tag="xt1", name="xt1"),
                    persist.tile([P, ND, 1024], CDT, tag="xt2", name="xt2")]

            def xt_ap(dt, a, b):
                if b <= 512:
                    return xt_c[0][:, dt, a:b]
                if b <= 1024:
                    return xt_c[1][:, dt, a - 512:b - 512]
                return xt_c[2][:, dt, a - 1024:b - 1024]

            wqk_sb = persist.tile([P, ND, 2 * HPC * Dh], CDT, tag="wqk")
            wv_sb = persist.tile([P, ND, HPC * Dh], CDT, tag="wv")
            wo_sb = persist.tile([P, NPAIR, D], CDT, tag="wo")
            pm_sb = persist.tile([P, n_pm, QT], CDT, tag="pm")
            scratch = persist.tile([P, QT], CDT, tag="scratch")
            # per-pair tensors (separate tiles so cross-pair interleaving
            # cannot create false dependencies)
            qT_sb = [persist.tile([P, T], CDT, tag=f"qT{p}", name=f"qT{p}") for p in range(NPAIR)]
            kT_sb = [persist.tile([P, T], CDT, tag=f"kT{p}", name=f"kT{p}") for p in range(NPAIR)]
            # at tiles are split per (pair, q-tile) so deferred out-projection
            # reads never serialize behind later q-tiles' normalization writes
            at_sb = [[persist.tile([P, QT], CDT, tag=f"at{p}q{q}", name=f"at{p}q{q}")
                      for q in range(NQ)] for p in range(NPAIR)]
            # v1: per k-tile and head, [128, 128]: for even heads cols 0:64 =
            # v values and cols 64:128 all-ones (for odd heads the reverse),
            # so the attnU matmul emits softmax denominators replicated on the
            # complementary partition half (matmul cost only depends on N).
            v1_sb = persist.tile([P, NKT, HPC, P], CDT, tag="v1")

            # ---- PE p-state warmup: garbage matmuls with no DMA deps ----
            nc.vector.memset(scratch[:], 0.0)
            ps_warm = psA.tile([P, QT], f32, tag="psA", name="warm")
            for i in range(N_WARM):
                nc.tensor.matmul(
                    ps_warm[:],
                    scratch[:, 0:P],
                    scratch[:],
                    start=(i == 0),
                    stop=(i == N_WARM - 1),
                )

            # ones blocks only (v halves written by emit_v)
            nc.vector.memset(v1_sb[:, :, 0::2, Dh:P], 1.0)
            nc.vector.memset(v1_sb[:, :, 1::2, 0:Dh], 1.0)

            # ---- DMAs in criticality order; flat APs so rows coalesce ----
            def flat(ap):
                return ap.rearrange("p a b -> p (a b)")

            nc.sync.dma_start(flat(wv_sb[:]), flat(wv[:]))
            nc.sync.dma_start(xt_c[0][:, :, 0:256], xc0a[:])
            nc.sync.dma_start(xt_c[0][:, :, 256:512], xc0b[:])
            nc.sync.dma_start(flat(wqk_sb[:]), flat(wqk[:]))
            nc.sync.dma_start(flat(pm_sb[:]), flat(pmask[:]))
            nc.sync.dma_start(flat(xt_c[1][:]), flat(xc1[:]))
            nc.sync.dma_start(flat(xt_c[2][:]), flat(xc2[:]))
            nc.sync.dma_start(flat(wo_sb[:]), flat(wo[:]))

            # ---- filler queue: PE-side work interleaved into ACT-paced ----
            # ---- attention steps                                        ----
            fillers = []  # list of (key, thunk); emitted in order
            late = []     # out-projections reserved to overlap the tail

            def drain(k):
                for _ in range(min(k, len(fillers))):
                    fillers.pop(0)[1]()

            def flush_through(pred):
                """Emit queued fillers (in order) until none matching pred remain."""
                while any(pred(key) for key, _ in fillers):
                    fillers.pop(0)[1]()

            # ---- phase A: v = x @ wv ----
            def emit_v(tt):
                ps_v = psA.tile([P, QT], f32, tag="psA", name=f"psv{tt}")
                for dt in range(ND):
                    nc.tensor.matmul(
                        ps_v[:, :HPC * Dh],
                        xt_ap(dt, tt * P, (tt + 1) * P),
                        wv_sb[:, dt, :],
                        start=(dt == 0),
                        stop=(dt == ND - 1),
                    )
                ps_vh = ps_v[:, :HPC * Dh].rearrange("p (h e) -> p h e", h=HPC)
                nc.vector.tensor_copy(v1_sb[:, tt, 0::2, 0:Dh], ps_vh[:, 0::2])
                nc.vector.tensor_copy(v1_sb[:, tt, 1::2, Dh:P], ps_vh[:, 1::2])

            # ---- phase A: qT / kT for pair p, one (tensor, nt) at a time ----
            def qk_units(p, nt):
                units = []
                # pair-major weight layout: [wq_p0 | wk_p0 | wq_p1 | wk_p1]
                for w_off, out_sb in ((p * 2 * P, qT_sb[p]), (p * 2 * P + P, kT_sb[p])):
                    ps_box = []

                    def mm(dt, w_off=w_off, nt=nt, ps_box=ps_box, p=p):
                        if dt == 0:
                            ps_box.append(psA.tile(
                                [P, QT], f32, tag="psA", name=f"psqk{p}_{nt}_{w_off}"))
                        nc.tensor.matmul(
                            ps_box[0],
                            wqk_sb[:, dt, w_off:w_off + P],
                            xt_ap(dt, nt * QT, (nt + 1) * QT),
                            start=(dt == 0),
                            stop=(dt == ND - 1),
                        )

                    def evict(out_sb=out_sb, nt=nt, ps_box=ps_box):
                        nc.vector.tensor_copy(
                            out_sb[:, nt * QT:(nt + 1) * QT], ps_box[0])

                    key = ("qk", p, nt)

                    def mk(dt, mm=mm):
                        return lambda: mm(dt)

                    units.extend((key, mk(dt)) for dt in range(ND))
                    units.append((key, evict))
                return units

            # ---- phase C: out-projection for one t-tile/half (as filler) ----
            def make_outproj(tt, half):
                def go():
                    ps_y = psA.tile([P, QT], f32, tag="psA", name=f"psy{tt}_{half}")
                    qi, ci = tt // (QT // P), (tt % (QT // P)) * P
                    for p in range(NPAIR):
                        nc.tensor.matmul(
                            ps_y[:],
                            at_sb[p][qi][:, ci:ci + P],
                            wo_sb[:, p, half * QT:(half + 1) * QT],
                            start=(p == 0),
                            stop=(p == NPAIR - 1),
                        )
                    ysb = work.tile([P, QT], CDT, tag="y", name=f"y{tt}_{half}")
                    nc.vector.tensor_copy(ysb[:], ps_y[:])
                    nc.sync.dma_start(
                        y[tt * P:(tt + 1) * P, half * QT:(half + 1) * QT], ysb[:])
                return go

            # ---- phase B: attention for pair p, q-tile qt (sw-pipelined) ----
            def emit_attention(p, qt):
                row = vis[qt]
                max_kt = max((kt for kt, _ in row), default=-1)
                flush_through(lambda key: (
                    (key[0] == "v" and key[1] <= max_kt)
                    or (key[0] == "qk" and key[1] == p and key[2] == qt)))
                if not row:
                    for h in range(2):
                        nc.vector.memset(at_sb[p][qt][h * Dh:(h + 1) * Dh, :], 0.0)
                    return
                ps_u = [
                    psU.tile([P, QT], f32, tag=f"u{h}", name=f"u{h}_{p}_{qt}")
                    for h in range(2)
                ]
                es_q = []

                def emit_scores(j, p=p, qt=qt, row=row, es_q=es_q):
                    kt, pidx = row[j]
                    w0 = 0 if pidx is None else meta[pidx][0]
                    if j == 0:
                        w0 = 0  # first accumulation must set has_written
                    # both heads' scores into one two-bank psum tile; the two
                    # matmuls run concurrently on complementary PE tile rows
                    ps_s = psS.tile([P, 2, QT], f32, tag="s", name=f"s_{p}_{qt}_{kt}")
                    for h in range(2):
                        base = h * Dh
                        nc.tensor.matmul(
                            ps_s[:, h, w0:QT],
                            kT_sb[p][base:base + Dh, kt * P:(kt + 1) * P],
                            qT_sb[p][base:base + Dh, qt * QT + w0:(qt + 1) * QT],
                            start=True,
                            stop=True,
                            tile_position=(base, 0),
                        )
                    es = work.tile([P, 2, QT], CDT, tag="es", bufs=5, name=f"es_{p}_{qt}_{kt}")
                    nc.scalar.activation(
                        es[:, :, w0:QT], ps_s[:, :, w0:QT],
                        mybir.ActivationFunctionType.Exp,
                        scale=inv_sqrt_dh,
                    )
                    if pidx is not None:
                        _w0, m_lo, m_hi = meta[pidx]
                        m_lo = min(m_lo, w0)  # w0 forced to 0 on j==0
                        if m_hi > m_lo:
                            # run the mask multiplies on the otherwise-idle
                            # GpSimd engine: keeps the exp->attnU dependency
                            # off the in-order DVE queue
                            for h in range(2):
                                nc.gpsimd.tensor_mul(
                                    es[:, h, m_lo:m_hi], es[:, h, m_lo:m_hi],
                                    pm_sb[:, pidx, m_lo:m_hi],
                                )
                    es_q.append((es, w0))

                def emit_attnu(j, p=p, row=row, es_q=es_q, ps_u=ps_u):
                    kt, _ = row[j]
                    es, w0 = es_q[j]
                    for h in range(2):
                        nc.tensor.matmul(
                            ps_u[h][:, w0:QT],
                            v1_sb[:, kt, 2 * p + h, :],
                            es[:, h, w0:QT],
                            start=(j == 0),
                            stop=(j == len(row) - 1),
                        )

                # process blocks in batches of two so the PE array switches
                # between tiled mode (64-row scores pairs) and full mode
                # (attnU + fillers) half as often — each mode switch exposes
                # a ~100ns weight-load/drain.  Fillers run on the full-mode
                # side of each batch.
                emit_scores(0)
                if len(row) > 1:
                    emit_scores(1)
                for b0 in range(0, len(row), 2):
                    cur = [j for j in (b0, b0 + 1) if j < len(row)]
                    nxt = [j for j in (b0 + 2, b0 + 3) if j < len(row)]
                    drain(2 * len(cur))
                    for j in nxt:
                        emit_scores(j)
                    for j in cur:
                        emit_attnu(j)

                # normalization: h0 denom replicated at psum partitions 64:128
                # of ps_u[0], h1 denom at partitions 0:64 of ps_u[1].  Evict
                # both psum tiles to SBUF immediately (frees psU for the next
                # pair; DVE copy cost is free-dim-bound so full-width copies
                # cost the same as halves), swap denominator halves via
                # sbuf-to-sbuf DMA, one full-width reciprocal, then one
                # aligned multiply per head.
                s0 = work.tile([P, QT], f32, tag="s0", name=f"s0_{p}_{qt}")
                s1 = work.tile([P, QT], f32, tag="s1", name=f"s1_{p}_{qt}")
                nc.vector.tensor_copy(s0[:], ps_u[0][:])
                nc.vector.tensor_copy(s1[:], ps_u[1][:])
                comb = work.tile([P, QT], f32, tag="comb", name=f"comb_{p}_{qt}")
                # dispatch the two half-swaps from different engine queues so
                # their descriptor-generation does not serialize
                nc.sync.dma_start(comb[0:Dh, :], s0[Dh:P, :])
                nc.scalar.dma_start(comb[Dh:P, :], s1[0:Dh, :])
                rep = work.tile([P, QT], f32, tag="rep", name=f"rep_{p}_{qt}")
                nc.vector.reciprocal_approx_fast(rep[:], comb[:])
                nc.vector.tensor_mul(at_sb[p][qt][0:Dh, :], s0[0:Dh, :], rep[0:Dh, :])
                nc.vector.tensor_mul(at_sb[p][qt][Dh:P, :], s1[Dh:P, :], rep[Dh:P, :])
                if p == NPAIR - 1:
                    # out-projection for the t-tiles this qt completed; the
                    # last two q-tiles' units are reserved so the PE has work
                    # overlapping the final normalization chain + teardown
                    _late = qt >= NQ - 3
                    dst = late if _late else fillers
                    dst.extend(
                        (("op", tt, half), make_outproj(tt, half))
                        for tt in range(qt * (QT // P), (qt + 1) * (QT // P))
                        for half in range(2)
                    )

            # inline prologue: just enough for attention(p0, qt0) to start
            for tt in range(NQ):
                emit_v(tt)
            for nt in range(NQ):
                for p in range(NPAIR):
                    fillers.extend(qk_units(p, nt))
                if nt + 1 < NQ:
                    lo, hi = (nt + 1) * NQ, (nt + 2) * NQ
                    fillers.extend(
                        (("v", tt), (lambda tt=tt: emit_v(tt)))
                        for tt in range(lo, min(hi, NKT)))
            for qt in range(NQ):
                for p in range(NPAIR):
                    emit_attention(p, qt)
            while fillers:
                drain(len(fillers))
            for _, thunk in late:
                thunk()
    if compile:
        nc.compile()
    return nc


def _host_inputs(x, mask, w_qkv, w_out):
    vis, pm, meta = _block_structure(np.asarray(mask))
    pm_c = pm.astype(NP_CDT)
    wq_f, wk_f, wv_f = np.split(np.asarray(w_qkv, np.float32), 3, axis=1)
    in_maps = []
    for core in range(N_CORES):
        b = core // 4
        g = core % 4
        cols = slice(g * HPC * Dh, (g + 1) * HPC * Dh)
        wq_c, wk_c = wq_f[:, cols], wk_f[:, cols]
        wqk_c = np.concatenate(
            [wq_c[:, 0:2 * Dh], wk_c[:, 0:2 * Dh],
             wq_c[:, 2 * Dh:], wk_c[:, 2 * Dh:]], axis=1)

        def pack_d(a):  # [D, E] -> [P, ND, E] matching d = o*P + p
            return np.ascontiguousarray(
                a.reshape(ND, P, -1).transpose(1, 0, 2)).astype(NP_CDT)

        xp = pack_d(np.ascontiguousarray(np.asarray(x[b], np.float32).T))
        in_maps.append({
            "xc0a": np.ascontiguousarray(xp[:, :, 0:256]),
            "xc0b": np.ascontiguousarray(xp[:, :, 256:512]),
            "xc1": np.ascontiguousarray(xp[:, :, 512:1024]),
            "xc2": np.ascontiguousarray(xp[:, :, 1024:2048]),
            "wqk": pack_d(wqk_c),
            "wv": pack_d(wv_f[:, cols]),
            "wo": np.ascontiguousarray(
                np.asarray(w_out, np.float32)[cols, :]
                .reshape(NPAIR, P, D).transpose(1, 0, 2)).astype(NP_CDT),
            "pmask": np.ascontiguousarray(pm_c.transpose(1, 0, 2)),
        })
    return vis, pm, meta, in_maps


def run(x, mask, w_qkv, w_out, trace=False):
    import os
    vis, pm, meta, in_maps = _host_inputs(x, mask, w_qkv, w_out)
    nc = _build_program(vis, pm.shape[0], meta)
    if not trace:
        # An inherited BASS_TRACE=1 would pull in NTFF profiling hooks that
        # may not exist in this environment; force tracing off.
        os.environ["BASS_NEVER_TRACE"] = "1"
    else:
        os.environ.pop("BASS_NEVER_TRACE", None)
    res = run_bass_kernel_spmd(nc, in_maps, core_ids=list(range(N_CORES)), trace=trace)
    parts = [res.results[i]["y"].astype(np.float32) for i in range(N_CORES)]
    out = np.stack([
        parts[0] + parts[1] + parts[2] + parts[3],
        parts[4] + parts[5] + parts[6] + parts[7],
    ]).astype(np.float32)
    return out, res


def kernel(x, mask, w_qkv, w_out):
    out, _ = run(x, mask, w_qkv, w_out, trace=False)
    return out
